# revision 3
# baseline (speedup 1.0000x reference)
"""Multi-head self-attention (B=4, S=2048, E=1024, H=16) + residual + layernorm
on 8 Trainium2 NeuronCores — fp8 DoubleRow, projection-overlapped, ACT+DVE
split-softmax version.

Sharding: data-parallel over batch (4) x query-split (2) = 8 cores, no
collectives; K/V duplicated across the query-split pair.

vs the previous fp8 baseline (457us measured / 480us same-session):
- Projections are interleaved into the attention pipeline: K/Q for heads
  0-1 up front, K/Q for head j+2 inside block j's q-loop, V s2t tiles
  inside blocks 0-1 — the ScalarE exp stream starts ~16us in instead of
  ~70us. Input DMAs are single 3D-descriptor transfers (48 per-kt chunks
  cost ~30us of serial HWDGE issue).
- The softmax exp (the bottleneck: 262k exps/partition, ~1.04us per
  [128,2,512] tile on ACT) is split between ACT and DVE: DVE computes the
  fp8e4m3 BIT PATTERN of exp(s) in one tensor_scalar
  (uint8 = st*8*log2e/16384 + 56.15, Schraudolph in the fp8 code domain)
  through a bitcast view. HW A/B: 0 DVE tiles -> 479us, 67 -> 471us,
  112 -> 509us (real DVE cost ~1.7us/tile vs 1.19 modeled), so ~67.
- pv_norm: bf16 reciprocal of the [1,512] denominator row, Pool
  partition_broadcast (no PE matmul, no PSUM bank), one DVE multiply;
  recips right after PV stops, broadcasts+muls early next block (the pv
  PSUM ring (2 banks) aliases block i with i+1).
- rstd = rsqrt(var) via the 0x5f3759df int trick + 2 Newton steps in tiny
  DVE ops: every ACT Ln<->Exp table flip reloads a 1.28us table.
- WO/LN epilogue in per-half closures popped every iteration; the tail
  (sb1=1) group runs per-pair chains with prefetched residuals so output
  DMAs start asap; output stores issue from the Pool queue (on the sync
  queue they head-of-line block the next rep's input DMA prefetch).
- Instruction count matters on HW (~0.2us/instr beyond the cost model:
  a 4-deep single-bank score-ring variant with +256 instrs measured
  +60us): V ones-columns are one broadcast write, DMAs consolidated.
- PSUM budget (8 banks): score ring 2x[128,2,512] (4) + pv 2x[65,512]
  (2) + matmul scratch 2x[128,512] (2).
"""
import numpy as np
import ml_dtypes

B, S, E = 4, 2048, 1024
H, D = 16, 64
SQ = S // 2
N_CORES = 8

_CACHE = {}

# DVE exp-offload schedule: number of the 16 s2t exp tiles per block
# assigned to DVE (rest go to ACT). Blocks 0-5 keep DVE free for the
# interleaved projection epilogues.
DVE_COUNTS = [0, 0, 4, 4, 4, 4, 5, 5, 6, 6, 5, 5, 5, 6, 6, 2]
# Pool/GPSIMD can't read PSUM and any staging copy costs the stager as much
# as the exp itself (ACT/DVE are element-bound), so no Pool exp offload.
POOL_COUNTS = [0] * 16
# fp8e4m3 exp bit trick: bits = round(8*(s*log2e + 7) + c)
EXP_A = 8.0 * 1.4426950408889634 / 16384.0
EXP_B = 56.0 + 0.15  # +0.5 truncation guess - 0.35 pwl centering
# pv_norm denominator broadcast: Pool partition_broadcast vs PE ones-matmul
USE_POOL_BCAST = True


def _spread(n):
    return {int((j + 0.5) * 16.0 / n) for j in range(n)} if n else set()


def _assign(nd, npool):
    """Spread nd DVE picks over 0..15, then npool Pool picks over the rest."""
    d = _spread(nd)
    rem = [s for s in range(16) if s not in d]
    p = ({rem[int((j + 0.5) * len(rem) / npool)] for j in range(npool)}
         if npool else set())
    return d, p


def _build_nc(unroll=1):
    import concourse.bass as bass
    import concourse.mybir as mybir
    import concourse.tile as tile
    from concourse import bacc

    F32 = mybir.dt.float32
    BF16 = mybir.dt.bfloat16
    FP8 = mybir.dt.float8e4
    U8 = mybir.dt.uint8
    AF = mybir.ActivationFunctionType
    DR = mybir.MatmulPerfMode.DoubleRow
    ALU = mybir.AluOpType

    nc = bacc.Bacc("TRN2", target_bir_lowering=False, debug=False,
                   num_devices=N_CORES)

    xT = nc.declare_dram_parameter("xT", [E, S], FP8, isOutput=False)
    xqT = nc.declare_dram_parameter("xqT", [E, SQ], FP8, isOutput=False)
    x_res = nc.declare_dram_parameter("x_res", [SQ, E], BF16, isOutput=False)
    wqT = nc.declare_dram_parameter("wqT", [E, E], FP8, isOutput=False)
    wkT = nc.declare_dram_parameter("wkT", [E, E], FP8, isOutput=False)
    wvT = nc.declare_dram_parameter("wvT", [E, E], FP8, isOutput=False)
    woT = nc.declare_dram_parameter("woT", [E, E], FP8, isOutput=False)
    bq = nc.declare_dram_parameter("bq", [128, 8], F32, isOutput=False)
    bk = nc.declare_dram_parameter("bk", [128, 8], F32, isOutput=False)
    em32 = nc.declare_dram_parameter("em32", [128, 16], F32, isOutput=False)
    ln_w_row = nc.declare_dram_parameter("ln_w_row", [1, E], BF16,
                                         isOutput=False)
    ln_b_row = nc.declare_dram_parameter("ln_b_row", [1, E], BF16,
                                         isOutput=False)

    out_half = nc.declare_dram_parameter("out_half", [SQ, E], BF16,
                                         isOutput=True)

    def bc_ap(param, n):
        return bass.AP(tensor=param, offset=0, ap=[[0, 128], [1, n]])

    with tile.TileContext(nc) as tc:
        with tc.tile_pool(name="persist", bufs=1) as pp, \
             tc.tile_pool(name="psum", bufs=2, space="PSUM") as ps, \
             tc.tile_pool(name="small", bufs=2) as sp:

          for _rep in range(unroll):
            pfx = f"r{_rep}_"

            # ---------- small constants ----------
            bq_t = pp.tile([128, 8], F32, tag="bq")
            nc.sync.dma_start(out=bq_t[:], in_=bq.ap())
            bk_t = pp.tile([128, 8], F32, tag="bk")
            nc.sync.dma_start(out=bk_t[:], in_=bk.ap())
            em_t = pp.tile([128, 16], F32, tag="em")
            nc.sync.dma_start(out=em_t[:], in_=em32.ap())
            ones_row = pp.tile([1, 64], BF16, tag="ones_row")
            nc.vector.memset(ones_row[:], 1.0)
            eps_t = pp.tile([128, 1], F32, tag="eps")
            nc.vector.memset(eps_t[:], 1e-12)

            # persistent activations
            q_t = pp.tile([128, 8, SQ], FP8, tag="Q")       # 32*q
            k_t = pp.tile([128, 8, S], FP8, tag="K")        # 32*k
            v_t = pp.tile([128, 16, 16, 65], FP8, tag="V")  # 32*em*v
            ctx_t = pp.tile([128, 8, SQ], FP8, tag="ctx")   # ctx (pv/den)
            # all V ones-columns at once: v_t[:, s2t, hl, 64] = 32*em[s2t]
            nc.gpsimd.tensor_scalar_mul(
                out=v_t[:, :, :, 64],
                in0=em_t[:, :].unsqueeze(2).broadcast_to((128, 16, 16)),
                scalar1=1.0)

            with tc.tile_pool(name="attn", bufs=1) as ap_pool, \
                 tc.tile_pool(name="epi", bufs=2) as ep:
                # ---- DMAs, in first-use order ----
                # single 3D-descriptor DMAs: 48 per-kt chunks cost
                # 48x625ns of serial HWDGE issue (~30us of lead-in)
                xT_t = ap_pool.tile([128, 8, S], FP8, tag="xT")
                nc.sync.dma_start(out=xT_t[:], in_=xT.ap().rearrange(
                    "(kt p) s -> p kt s", p=128))
                wk_t = ap_pool.tile([128, 8, E], FP8, tag="wk")
                nc.sync.dma_start(out=wk_t[:], in_=wkT.ap().rearrange(
                    "(kt p) m -> p kt m", p=128))
                xqT_t = ap_pool.tile([128, 8, SQ], FP8, tag="xqT")
                nc.sync.dma_start(out=xqT_t[:], in_=xqT.ap().rearrange(
                    "(kt p) s -> p kt s", p=128))
                wq_t = ap_pool.tile([128, 8, E], FP8, tag="wq")
                nc.sync.dma_start(out=wq_t[:], in_=wqT.ap().rearrange(
                    "(kt p) m -> p kt m", p=128))
                wv_t = ap_pool.tile([128, 8, E], FP8, tag="wv")
                nc.sync.dma_start(out=wv_t[:], in_=wvT.ap().rearrange(
                    "(kt p) m -> p kt m", p=128))
                wo_t = ap_pool.tile([128, 8, E], FP8, tag="wo")
                nc.sync.dma_start(out=wo_t[:], in_=woT.ap().rearrange(
                    "(mt p) eo -> p mt eo", p=128))
                lnw_bc = ap_pool.tile([128, E], BF16, tag="lnw_bc")
                nc.sync.dma_start(out=lnw_bc[:], in_=bc_ap(ln_w_row, E))
                lnb_bc = ap_pool.tile([128, E], BF16, tag="lnb_bc")
                nc.sync.dma_start(out=lnb_bc[:], in_=bc_ap(ln_b_row, E))

                # ---- projection emitters ----
                def emit_k_group(mt, sb):
                    p = ps.tile([128, 512], F32, tag="mm", bufs=2,
                                name=f"kp{pfx}{mt}_{sb}")
                    for q in range(4):
                        nc.tensor.matmul(
                            p[:],
                            wk_t[:, 2 * q:2 * q + 2,
                                 mt * 128:(mt + 1) * 128],
                            xT_t[:, 2 * q:2 * q + 2,
                                 sb * 512:(sb + 1) * 512],
                            start=(q == 0), stop=(q == 3), perf_mode=DR)
                    nc.vector.tensor_scalar_add(
                        out=k_t[:, mt, sb * 512:(sb + 1) * 512],
                        in0=p[:], scalar1=bk_t[:, mt:mt + 1])

                def emit_q_group(mt, sb):
                    p = ps.tile([128, 512], F32, tag="mm", bufs=2,
                                name=f"qp{pfx}{mt}_{sb}")
                    for q in range(4):
                        nc.tensor.matmul(
                            p[:],
                            wq_t[:, 2 * q:2 * q + 2,
                                 mt * 128:(mt + 1) * 128],
                            xqT_t[:, 2 * q:2 * q + 2,
                                  sb * 512:(sb + 1) * 512],
                            start=(q == 0), stop=(q == 3), perf_mode=DR)
                    nc.vector.tensor_scalar_add(
                        out=q_t[:, mt, sb * 512:(sb + 1) * 512],
                        in0=p[:], scalar1=bq_t[:, mt:mt + 1])

                def emit_v_proj(s2t):
                    # v_t = (psum*0.5)*em32  (=32*em*v)
                    for half in range(2):
                        p = ps.tile([128, 512], F32, tag="mm", bufs=2,
                                    name=f"vp{pfx}{s2t}_{half}")
                        for q in range(4):
                            nc.tensor.matmul(
                                p[:],
                                xT_t[:, 2 * q:2 * q + 2,
                                     s2t * 128:(s2t + 1) * 128],
                                wv_t[:, 2 * q:2 * q + 2,
                                     half * 512:(half + 1) * 512],
                                start=(q == 0), stop=(q == 3), perf_mode=DR)
                        # psum=64*v; (64v * 1/64) * (32*em) = 32*em*v
                        nc.vector.tensor_scalar(
                            out=v_t[:, s2t, half * 8:(half + 1) * 8, 0:64],
                            in0=p[:].rearrange("p (h d) -> p h d", h=8),
                            scalar1=1.0 / 64.0, scalar2=em_t[:, s2t:s2t + 1],
                            op0=ALU.mult, op1=ALU.mult)

                # K/Q for heads 0-1 up front; the rest interleave below.
                for hm in (0, 1):
                    for sb in range(4):
                        emit_k_group(hm, sb)
                    for sb in range(2):
                        emit_q_group(hm, sb)

                # ---- attention pipeline ----
                blocks = [(sb1, hm) for sb1 in range(2) for hm in range(8)]
                sets = [_assign(nd, npool) for nd, npool
                        in zip(DVE_COUNTS, POOL_COUNTS)]
                dve_sets = [s[0] for s in sets]
                pool_sets = [s[1] for s in sets]
                state = {}

                # extra PE work interleaved into block i's q-loop:
                # extra[i][q] = list of closures (projections).
                extra = [[[] for _ in range(8)] for _ in range(16)]
                for qq in range(8):
                    extra[0][qq].append(lambda s2t=qq: emit_v_proj(s2t))
                    extra[1][qq].append(lambda s2t=8 + qq: emit_v_proj(s2t))
                for j in range(6):  # K/Q of head j+2 during block j
                    hm = j + 2
                    for g in range(4):
                        extra[j][g].append(
                            lambda mt=hm, sb=g: emit_k_group(mt, sb))
                    for g in range(2):
                        extra[j][4 + g].append(
                            lambda mt=hm, sb=g: emit_q_group(mt, sb))

                def emit_scores_half(i, s2t):
                    # fp8 DoubleRow, stride-0 slot dims: psum = 2048*q.k
                    sb1, hm = blocks[i]
                    st = ps.tile([128, 2, 512], F32, tag="st", bufs=2,
                                 name=f"st{pfx}{i}_{s2t}")
                    s1 = slice(sb1 * 512, (sb1 + 1) * 512)
                    for idx, hp in enumerate((0, 64)):
                        lh = k_t[hp:hp + 64, hm,
                                 s2t * 128:(s2t + 1) * 128].unsqueeze(
                                     1).broadcast_to((64, 2, 128))
                        rh = q_t[hp:hp + 64, hm, s1].unsqueeze(
                            1).broadcast_to((64, 2, 512))
                        nc.tensor.matmul(
                            st[:, idx, :], lh, rh,
                            start=True, stop=True, perf_mode=DR,
                            tile_position=(hp, 0))
                    exp_pair = state[i]["exp"]
                    if s2t in dve_sets[i]:
                        # fp8 bits of exp(st/16384) via one DVE op
                        nc.vector.tensor_scalar(
                            out=exp_pair[:, s2t, :, :].bitcast(U8),
                            in0=st[:], scalar1=EXP_A, scalar2=EXP_B,
                            op0=ALU.mult, op1=ALU.add)
                    elif s2t in pool_sets[i]:
                        # ACT stages PSUM->SBUF with a cheap Copy (570ns vs
                        # 1026ns for the exp), then the same bit-trick on
                        # the Pool/GPSIMD engine (which cannot read PSUM)
                        stg = sp.tile([128, 2, 512], F32, tag="stg", bufs=3,
                                      name=f"stg{pfx}{i}_{s2t}")
                        nc.scalar.activation(out=stg[:], in_=st[:],
                                             func=AF.Copy)
                        nc.gpsimd.tensor_scalar(
                            out=exp_pair[:, s2t, :, :].bitcast(U8),
                            in0=stg[:], scalar1=EXP_A, scalar2=EXP_B,
                            op0=ALU.mult, op1=ALU.add)
                    else:
                        nc.scalar.activation(
                            out=exp_pair[:, s2t, :, :], in_=st[:],
                            func=AF.Exp, scale=1.0 / 16384.0)

                def emit_pv_quad(i, q):
                    exp_pair = state[i]["exp"]
                    pvs = state[i]["pv"]
                    for idx in range(2):
                        hl = blocks[i][1] * 2 + idx
                        nc.tensor.matmul(
                            pvs[idx][:],
                            v_t[:, 2 * q:2 * q + 2, hl, :],
                            exp_pair[:, 2 * q:2 * q + 2, idx, :],
                            start=(q == 0), stop=(q == 7), perf_mode=DR)

                def emit_pv_recips(i):
                    # 1/den rows right after PV(i) stops; consumed a few
                    # iterations later so the Pool broadcast never waits.
                    for idx in range(2):
                        rec = sp.tile([1, 512], BF16, tag="rec", bufs=4,
                                      name=f"rec{pfx}{i}_{idx}")
                        with nc.allow_low_precision(
                                reason="bf16 1/den: 0.4% scale noise ok"):
                            nc.vector.reciprocal(
                                out=rec[:], in_=state[i]["pv"][idx][64:65, :])
                        state[i][f"rec{idx}"] = rec

                def emit_pv_finish(i, idx):
                    # Broadcast 1/den across partitions, then DVE
                    # multiplies: ctx = pv/den.
                    sb1, hm = blocks[i]
                    s1 = slice(sb1 * 512, (sb1 + 1) * 512)
                    hp = (0, 64)[idx]
                    pv = state[i]["pv"][idx]
                    if USE_POOL_BCAST:
                        rbc = sp.tile([64, 512], BF16, tag="rbc", bufs=2,
                                      name=f"rbc{pfx}{i}_{idx}")
                        nc.gpsimd.partition_broadcast(
                            rbc[:], state[i][f"rec{idx}"][:])
                        nc.vector.tensor_mul(
                            out=ctx_t[hp:hp + 64, hm, s1],
                            in0=pv[0:64, :], in1=rbc[:])
                    else:
                        bcp = ps.tile([64, 512], F32, tag="mm", bufs=2,
                                      name=f"bcp{pfx}{i}_{idx}")
                        nc.tensor.matmul(bcp[:], ones_row[:],
                                         state[i][f"rec{idx}"][:],
                                         start=True, stop=True)
                        nc.vector.tensor_mul(
                            out=ctx_t[hp:hp + 64, hm, s1],
                            in0=pv[0:64, :], in1=bcp[:])

                def wo_ln_tile_closures(sb1):
                    # sb1=0 (mid-stream): batched rstd + LN apply on the
                    # idle Pool engine. sb1=1 (tail): per-tile A->rstd->B
                    # chains on DVE so output DMAs start asap (no 4-tile
                    # barrier while ACT/DVE drain).
                    gmv = ep.tile([128, 4, 2], F32, tag="gmv",
                                  name=f"gmv{pfx}{sb1}")
                    rstd_g = ep.tile([128, 4], F32, tag="rstdg",
                                     name=f"rstdg{pfx}{sb1}")
                    vs = [ep.tile([128, E], BF16, tag="v", bufs=5,
                                  name=f"v{pfx}{sb1 * 4 + ti}")
                          for ti in range(4)]
                    # prefetch residual rows for the whole group
                    xrs = [ep.tile([128, E], BF16, tag="xr", bufs=4,
                                   name=f"xr{pfx}{sb1 * 4 + ti}")
                           for ti in range(4)]
                    for ti in range(4):
                        rows = slice((sb1 * 4 + ti) * 128,
                                     (sb1 * 4 + ti + 1) * 128)
                        nc.sync.dma_start(out=xrs[ti][:],
                                          in_=x_res.ap()[rows, :])
                    stats = [ep.tile([128, 2, 6], F32, tag="stats", bufs=4,
                                     name=f"stats{pfx}{sb1 * 4 + ti}")
                             for ti in range(4)]
                    cl = []
                    for ti in range(4):
                        for eb in range(2):
                            cl.append(lambda ti=ti, eb=eb: emit_wo_half_a(
                                sb1 * 4 + ti, ti, eb, gmv, vs[ti], xrs[ti],
                                stats[ti]))
                        if sb1 == 1 and ti % 2 == 1:
                            # pair-batched rstd (each Ln<->Exp flip costs an
                            # ACT table reload) then LN-apply both tiles;
                            # first pair on Pool (idle at the tail), last
                            # pair on DVE (shortest final chain)
                            eng = nc.gpsimd if ti == 1 else nc.vector
                            cl.append(lambda t0=ti - 1: emit_rstd_dve(
                                gmv, rstd_g, t0, 2, sb1))
                            for tj in (ti - 1, ti):
                                cl.append(lambda tj=tj, eng=eng:
                                          emit_ln_tile_b(
                                              sb1 * 4 + tj, tj, gmv, rstd_g,
                                              vs[tj], eng))
                    if sb1 == 0:
                        cl.insert(8, lambda: None)  # spacing no-op
                        cl.append(lambda: emit_rstd_dve(gmv, rstd_g, 0, 4,
                                                        sb1))
                        cl.extend([lambda ti=ti: emit_ln_tile_b(
                            sb1 * 4 + ti, ti, gmv, rstd_g, vs[ti],
                            nc.vector) for ti in range(4)])
                    return cl

                def emit_wo_half_a(st_i, ti, eb, gmv, v, xr, stats):
                    p = ps.tile([128, 512], F32, tag="mm", bufs=2,
                                name=f"wop{pfx}{st_i}_{eb}")
                    for m in range(4):
                        nc.tensor.matmul(
                            p[:],
                            ctx_t[:, 2 * m:2 * m + 2,
                                  st_i * 128:(st_i + 1) * 128],
                            wo_t[:, 2 * m:2 * m + 2,
                                 eb * 512:(eb + 1) * 512],
                            start=(m == 0), stop=(m == 3), perf_mode=DR)
                    # v = psum/64 + x_res'  (ctx unscaled, WO host-scale 64)
                    nc.vector.scalar_tensor_tensor(
                        out=v[:, eb * 512:(eb + 1) * 512], in0=p[:],
                        scalar=1.0 / 64.0,
                        in1=xr[:, eb * 512:(eb + 1) * 512],
                        op0=ALU.mult, op1=ALU.add)
                    nc.vector.bn_stats(out=stats[:, eb, :],
                                       in_=v[:, eb * 512:(eb + 1) * 512])
                    if eb == 1:
                        nc.vector.bn_aggr(out=gmv[:, ti, :], in_=stats[:])

                def emit_rstd_dve(gmv, rstd_g, t0, n, sb1):
                    # rstd = rsqrt(var) via the int bit trick + 2 Newton
                    # steps, all tiny [128,n] DVE ops: keeps ACT on the
                    # softmax Exp table (each Ln/Exp flip reloads a table,
                    # 1.28us) and off the tail critical path.
                    I32 = mybir.dt.int32
                    var = gmv[:, t0:t0 + n, 1]
                    w = sp.tile([128, 4], I32, tag="rsq_i",
                                name=f"rsqi{pfx}{sb1}_{t0}")[:, 0:n]
                    nc.vector.tensor_scalar(
                        out=w, in0=var.bitcast(I32),
                        scalar1=1, scalar2=None,
                        op0=ALU.arith_shift_right)
                    nc.vector.tensor_scalar(
                        out=w, in0=w, scalar1=-1, op0=ALU.mult,
                        scalar2=0x5f3759df, op1=ALU.add)
                    y = w.bitcast(F32)
                    t = sp.tile([128, 4], F32, tag="rsq_t",
                                name=f"rsqt{pfx}{sb1}_{t0}")[:, 0:n]
                    for it in range(2):
                        dst = rstd_g[:, t0:t0 + n] if it == 1 else None
                        nc.vector.tensor_mul(out=t, in0=y, in1=y)
                        nc.vector.tensor_mul(out=t, in0=t, in1=var)
                        nc.vector.tensor_scalar(
                            out=t, in0=t, scalar1=-0.5, op0=ALU.mult,
                            scalar2=1.5, op1=ALU.add)
                        if it == 0:
                            y2 = sp.tile([128, 4], F32, tag="rsq_y",
                                         name=f"rsqy{pfx}{sb1}_{t0}")[:, 0:n]
                            nc.vector.tensor_mul(out=y2, in0=y, in1=t)
                            y = y2
                        else:
                            nc.vector.tensor_mul(out=dst, in0=y, in1=t)

                def emit_ln_tile_b(st_i, ti, gmv, rstd_g, v, eng):
                    rows = slice(st_i * 128, (st_i + 1) * 128)
                    u = v  # in-place LN apply, bf16 (4x DVE mode on vector)
                    eng.tensor_scalar(
                        out=u[:], in0=v[:],
                        scalar1=gmv[:, ti, 0:1], scalar2=rstd_g[:, ti:ti + 1],
                        op0=ALU.subtract, op1=ALU.mult)
                    eng.tensor_mul(out=u[:], in0=u[:], in1=lnw_bc[:])
                    eng.tensor_add(out=u[:], in0=u[:], in1=lnb_bc[:])
                    # issue output stores from the Pool queue: on the sync
                    # queue they head-of-line block the next rep's input
                    # DMA prefetch at the unroll boundary
                    nc.gpsimd.dma_start(out=out_half.ap()[rows, :], in_=u[:])

                wo_queue = []
                for i in range(len(blocks) + 1):
                    if i < len(blocks):
                        state[i] = {
                            "exp": ap_pool.tile([128, 16, 2, 512], FP8,
                                                tag="exp", bufs=2,
                                                name=f"exp{pfx}{i}"),
                            "pv": [ps.tile([65, 512], F32, tag="pv", bufs=2,
                                           name=f"pv{pfx}{i}_{idx}")
                                   for idx in range(2)],
                        }
                    for q in range(8):
                        if i < len(blocks):
                            emit_scores_half(i, 2 * q)
                            emit_scores_half(i, 2 * q + 1)
                            for fn in extra[i][q]:
                                fn()
                        if q == 0 and i >= 2 and i - 2 in state:
                            # both finishes before pv_quad(i-1, q0): the pv
                            # ring (2) aliases block i-2 with block i-1's
                            # accumulation running this loop
                            emit_pv_finish(i - 2, 0)
                            emit_pv_finish(i - 2, 1)
                            state.pop(i - 2)
                        if i > 0:
                            emit_pv_quad(i - 1, q)
                        if wo_queue:
                            wo_queue.pop(0)()
                    if i > 0:
                        emit_pv_recips(i - 1)
                        # ctx for blocks[i-2] completed this loop; enqueue
                        # the WO/LN group once its last head's ctx is done
                        if i >= 2 and blocks[i - 2][1] == 7:
                            wo_queue.extend(
                                wo_ln_tile_closures(blocks[i - 2][0]))
                last = len(blocks) - 1
                emit_pv_finish(last, 0)
                emit_pv_finish(last, 1)
                state.pop(last)
                wo_queue.extend(wo_ln_tile_closures(1))
                for fn in wo_queue:
                    fn()

    nc.finalize()
    return nc


def _prepare_in_maps(inputs):
    f8 = ml_dtypes.float8_e4m3
    bf = ml_dtypes.bfloat16
    f32 = np.float32
    x = np.ascontiguousarray(inputs["input_tensor"], dtype=f32)
    mask = np.ascontiguousarray(inputs["mask"], dtype=f32)
    WS = 64.0    # host weight pre-scale into fp8 range (V, O)
    WSQK = 32.0  # Q/K scale: 32*q stays under fp8 e4m3 max (240)
    res_bias = (np.asarray(inputs["WO_b"], f32)
                + np.asarray(inputs["WV_b"], f32)
                @ np.asarray(inputs["WO_w"], f32).T).reshape(1, E)
    in_maps = []
    for c in range(N_CORES):
        b, hc = divmod(c, 2)
        m = {
            "xT": np.ascontiguousarray(x[b].T).astype(f8),
            "xqT": np.ascontiguousarray(
                x[b, hc * SQ:(hc + 1) * SQ].T).astype(f8),
            "x_res": (x[b, hc * SQ:(hc + 1) * SQ] + res_bias).astype(bf),
            "wqT": np.ascontiguousarray(
                inputs["WQ_w"].T * WSQK).astype(f8),
            "wkT": np.ascontiguousarray(
                inputs["WK_w"].T * WSQK).astype(f8),
            "wvT": np.ascontiguousarray(
                inputs["WV_w"].T * WS).astype(f8),
            "woT": np.ascontiguousarray(
                inputs["WO_w"].T * WS).astype(f8),
            "bq": np.ascontiguousarray(
                (np.asarray(inputs["WQ_b"], f32) * WSQK).reshape(8, 128).T),
            "bk": np.ascontiguousarray(
                (np.asarray(inputs["WK_b"], f32) * WSQK).reshape(8, 128).T),
            "em32": np.ascontiguousarray(
                32.0 * np.exp(mask[b, 0, 0]).reshape(16, 128).T.astype(f32)),
            "ln_w_row": np.asarray(
                inputs["ln_w"], f32).reshape(1, E).astype(bf),
            "ln_b_row": np.asarray(
                inputs["ln_b"], f32).reshape(1, E).astype(bf),
        }
        in_maps.append({k: np.ascontiguousarray(v) for k, v in m.items()})
    return in_maps


def _run(inputs, trace=False):
    from concourse.bass_utils import run_bass_kernel_spmd

    if "nc" not in _CACHE:
        _CACHE["nc"] = _build_nc()
    in_maps = _prepare_in_maps(inputs)
    res = run_bass_kernel_spmd(_CACHE["nc"], in_maps, list(range(N_CORES)),
                               trace=trace)
    out = np.empty((B, S, E), np.float32)
    for c in range(N_CORES):
        b, hc = divmod(c, 2)
        out[b, hc * SQ:(hc + 1) * SQ] = res.results[c]["out_half"].astype(
            np.float32)
    return out, res


def kernel(**inputs):
    out, _ = _run(inputs, trace=False)
    return out


# revision 5
# speedup vs baseline: 1.0686x; 1.0686x over previous
"""Multi-head self-attention (B=4, S=2048, E=1024, H=16) + residual + layernorm
on 8 Trainium2 NeuronCores — fp8 DoubleRow, projection-overlapped, ACT+DVE
split-softmax version.

Sharding: data-parallel over batch (4) x query-split (2) = 8 cores, no
collectives; K/V duplicated across the query-split pair.

vs the previous fp8 baseline (457us measured / 480us same-session):
- Projections are interleaved into the attention pipeline: K/Q for heads
  0-1 up front, K/Q for head j+2 inside block j's q-loop, V s2t tiles
  inside blocks 0-1 — the ScalarE exp stream starts ~16us in instead of
  ~70us. Input DMAs are single 3D-descriptor transfers (48 per-kt chunks
  cost ~30us of serial HWDGE issue).
- The softmax exp (the bottleneck: 262k exps/partition, ~1.04us per
  [128,2,512] tile on ACT) is split between ACT and DVE: DVE computes the
  fp8e4m3 BIT PATTERN of exp(s) in one tensor_scalar
  (uint8 = st*8*log2e/16384 + 56.15, Schraudolph in the fp8 code domain)
  through a bitcast view. HW A/B: 0 DVE tiles -> 479us, 67 -> 471us,
  112 -> 509us (real DVE cost ~1.7us/tile vs 1.19 modeled), so ~67.
- pv_norm: bf16 reciprocal of the [1,512] denominator row, Pool
  partition_broadcast (no PE matmul, no PSUM bank), one DVE multiply;
  recips right after PV stops, broadcasts+muls early next block (the pv
  PSUM ring (2 banks) aliases block i with i+1).
- rstd = rsqrt(var) via the 0x5f3759df int trick + 2 Newton steps in tiny
  DVE ops: every ACT Ln<->Exp table flip reloads a 1.28us table.
- WO/LN epilogue in per-half closures popped every iteration; the tail
  (sb1=1) group runs per-pair chains with prefetched residuals so output
  DMAs start asap; output stores issue from the Pool queue (on the sync
  queue they head-of-line block the next rep's input DMA prefetch).
- Instruction count matters on HW (~0.2us/instr beyond the cost model:
  a 4-deep single-bank score-ring variant with +256 instrs measured
  +60us): V ones-columns are one broadcast write, DMAs consolidated.
- PSUM budget (8 banks): score ring 2x[128,2,512] (4) + pv 2x[65,512]
  (2) + matmul scratch 2x[128,512] (2).
"""
import numpy as np
import ml_dtypes

B, S, E = 4, 2048, 1024
H, D = 16, 64
SQ = S // 2
N_CORES = 8

_CACHE = {}

# DVE exp-offload schedule: number of the 16 s2t exp tiles per block
# assigned to DVE (rest go to ACT). Blocks 0-5 keep DVE free for the
# interleaved projection epilogues.
DVE_COUNTS = [0, 0, 3, 3, 3, 3, 4, 4, 5, 5, 4, 4, 4, 5, 5, 2]
# Pool/GPSIMD can't read PSUM and any staging copy costs the stager as much
# as the exp itself (ACT/DVE are element-bound), so no Pool exp offload.
POOL_COUNTS = [0] * 16
# fp8e4m3 exp bit trick: bits = round(8*(s*log2e + 7) + c)
EXP_A = 8.0 * 1.4426950408889634 / 16384.0
EXP_B = 56.0 + 0.15  # +0.5 truncation guess - 0.35 pwl centering
# pv_norm denominator broadcast: Pool partition_broadcast vs PE ones-matmul
USE_POOL_BCAST = True


def _spread(n):
    return {int((j + 0.5) * 16.0 / n) for j in range(n)} if n else set()


def _assign(nd, npool):
    """Spread nd DVE picks over 0..15, then npool Pool picks over the rest."""
    d = _spread(nd)
    rem = [s for s in range(16) if s not in d]
    p = ({rem[int((j + 0.5) * len(rem) / npool)] for j in range(npool)}
         if npool else set())
    return d, p


def _build_nc(unroll=1):
    import concourse.bass as bass
    import concourse.mybir as mybir
    import concourse.tile as tile
    from concourse import bacc

    F32 = mybir.dt.float32
    BF16 = mybir.dt.bfloat16
    FP8 = mybir.dt.float8e4
    U8 = mybir.dt.uint8
    AF = mybir.ActivationFunctionType
    DR = mybir.MatmulPerfMode.DoubleRow
    ALU = mybir.AluOpType

    nc = bacc.Bacc("TRN2", target_bir_lowering=False, debug=False,
                   num_devices=N_CORES)

    xT = nc.declare_dram_parameter("xT", [E, S], FP8, isOutput=False)
    xqT = nc.declare_dram_parameter("xqT", [E, SQ], FP8, isOutput=False)
    x_res = nc.declare_dram_parameter("x_res", [SQ, E], BF16, isOutput=False)
    wqT = nc.declare_dram_parameter("wqT", [E, E], FP8, isOutput=False)
    wkT = nc.declare_dram_parameter("wkT", [E, E], FP8, isOutput=False)
    wvT = nc.declare_dram_parameter("wvT", [E, E], FP8, isOutput=False)
    woT = nc.declare_dram_parameter("woT", [E, E], FP8, isOutput=False)
    bq = nc.declare_dram_parameter("bq", [128, 8], F32, isOutput=False)
    bk = nc.declare_dram_parameter("bk", [128, 8], F32, isOutput=False)
    em32 = nc.declare_dram_parameter("em32", [128, 16], F32, isOutput=False)
    ln_w_row = nc.declare_dram_parameter("ln_w_row", [1, E], BF16,
                                         isOutput=False)
    ln_b_row = nc.declare_dram_parameter("ln_b_row", [1, E], BF16,
                                         isOutput=False)

    out_half = nc.declare_dram_parameter("out_half", [SQ, E], BF16,
                                         isOutput=True)

    def bc_ap(param, n):
        return bass.AP(tensor=param, offset=0, ap=[[0, 128], [1, n]])

    with tile.TileContext(nc) as tc:
        with tc.tile_pool(name="persist", bufs=1) as pp, \
             tc.tile_pool(name="psum", bufs=2, space="PSUM") as ps, \
             tc.tile_pool(name="small", bufs=2) as sp:

          for _rep in range(unroll):
            pfx = f"r{_rep}_"

            # ---------- small constants ----------
            bq_t = pp.tile([128, 8], F32, tag="bq")
            nc.sync.dma_start(out=bq_t[:], in_=bq.ap())
            bk_t = pp.tile([128, 8], F32, tag="bk")
            nc.sync.dma_start(out=bk_t[:], in_=bk.ap())
            em_t = pp.tile([128, 16], F32, tag="em")
            nc.sync.dma_start(out=em_t[:], in_=em32.ap())
            if not USE_POOL_BCAST:
                ones_row = pp.tile([1, 64], BF16, tag="ones_row")
                nc.vector.memset(ones_row[:], 1.0)

            # persistent activations
            q_t = pp.tile([128, 8, SQ], FP8, tag="Q")       # 32*q
            k_t = pp.tile([128, 8, S], FP8, tag="K")        # 32*k
            v_t = pp.tile([128, 16, 16, 65], FP8, tag="V")  # 32*em*v
            ctx_t = pp.tile([128, 8, SQ], FP8, tag="ctx")   # ctx (pv/den)
            # all V ones-columns at once: v_t[:, s2t, hl, 64] = 32*em[s2t]
            nc.gpsimd.tensor_scalar_mul(
                out=v_t[:, :, :, 64],
                in0=em_t[:, :].unsqueeze(2).broadcast_to((128, 16, 16)),
                scalar1=1.0)

            with tc.tile_pool(name="attn", bufs=1) as ap_pool, \
                 tc.tile_pool(name="epi", bufs=2) as ep:
                # ---- DMAs, in first-use order ----
                # single 3D-descriptor DMAs: 48 per-kt chunks cost
                # 48x625ns of serial HWDGE issue (~30us of lead-in)
                xT_t = ap_pool.tile([128, 8, S], FP8, tag="xT")
                nc.sync.dma_start(out=xT_t[:], in_=xT.ap().rearrange(
                    "(kt p) s -> p kt s", p=128))
                wk_t = ap_pool.tile([128, 8, E], FP8, tag="wk")
                nc.sync.dma_start(out=wk_t[:], in_=wkT.ap().rearrange(
                    "(kt p) m -> p kt m", p=128))
                xqT_t = ap_pool.tile([128, 8, SQ], FP8, tag="xqT")
                nc.sync.dma_start(out=xqT_t[:], in_=xqT.ap().rearrange(
                    "(kt p) s -> p kt s", p=128))
                wq_t = ap_pool.tile([128, 8, E], FP8, tag="wq")
                nc.sync.dma_start(out=wq_t[:], in_=wqT.ap().rearrange(
                    "(kt p) m -> p kt m", p=128))
                wv_t = ap_pool.tile([128, 8, E], FP8, tag="wv")
                nc.sync.dma_start(out=wv_t[:], in_=wvT.ap().rearrange(
                    "(kt p) m -> p kt m", p=128))
                wo_t = ap_pool.tile([128, 8, E], FP8, tag="wo")
                nc.sync.dma_start(out=wo_t[:], in_=woT.ap().rearrange(
                    "(mt p) eo -> p mt eo", p=128))
                lnw_bc = ap_pool.tile([128, E], BF16, tag="lnw_bc")
                nc.sync.dma_start(out=lnw_bc[:], in_=bc_ap(ln_w_row, E))
                lnb_bc = ap_pool.tile([128, E], BF16, tag="lnb_bc")
                nc.sync.dma_start(out=lnb_bc[:], in_=bc_ap(ln_b_row, E))

                # ---- projection emitters ----
                def emit_k_group(mt, sb):
                    p = ps.tile([128, 512], F32, tag="mm", bufs=2,
                                name=f"kp{pfx}{mt}_{sb}")
                    for q in range(4):
                        nc.tensor.matmul(
                            p[:],
                            wk_t[:, 2 * q:2 * q + 2,
                                 mt * 128:(mt + 1) * 128],
                            xT_t[:, 2 * q:2 * q + 2,
                                 sb * 512:(sb + 1) * 512],
                            start=(q == 0), stop=(q == 3), perf_mode=DR)
                    nc.vector.tensor_scalar_add(
                        out=k_t[:, mt, sb * 512:(sb + 1) * 512],
                        in0=p[:], scalar1=bk_t[:, mt:mt + 1])

                def emit_q_group(mt, sb):
                    p = ps.tile([128, 512], F32, tag="mm", bufs=2,
                                name=f"qp{pfx}{mt}_{sb}")
                    for q in range(4):
                        nc.tensor.matmul(
                            p[:],
                            wq_t[:, 2 * q:2 * q + 2,
                                 mt * 128:(mt + 1) * 128],
                            xqT_t[:, 2 * q:2 * q + 2,
                                  sb * 512:(sb + 1) * 512],
                            start=(q == 0), stop=(q == 3), perf_mode=DR)
                    nc.vector.tensor_scalar_add(
                        out=q_t[:, mt, sb * 512:(sb + 1) * 512],
                        in0=p[:], scalar1=bq_t[:, mt:mt + 1])

                def emit_v_proj(s2t):
                    # v_t = (psum*0.5)*em32  (=32*em*v)
                    for half in range(2):
                        p = ps.tile([128, 512], F32, tag="mm", bufs=2,
                                    name=f"vp{pfx}{s2t}_{half}")
                        for q in range(4):
                            nc.tensor.matmul(
                                p[:],
                                xT_t[:, 2 * q:2 * q + 2,
                                     s2t * 128:(s2t + 1) * 128],
                                wv_t[:, 2 * q:2 * q + 2,
                                     half * 512:(half + 1) * 512],
                                start=(q == 0), stop=(q == 3), perf_mode=DR)
                        # psum=64*v; (64v * 1/64) * (32*em) = 32*em*v
                        nc.vector.tensor_scalar(
                            out=v_t[:, s2t, half * 8:(half + 1) * 8, 0:64],
                            in0=p[:].rearrange("p (h d) -> p h d", h=8),
                            scalar1=1.0 / 64.0, scalar2=em_t[:, s2t:s2t + 1],
                            op0=ALU.mult, op1=ALU.mult)

                # K/Q for heads 0-1 up front; the rest interleave below.
                for hm in (0, 1):
                    for sb in range(4):
                        emit_k_group(hm, sb)
                    for sb in range(2):
                        emit_q_group(hm, sb)

                # ---- attention pipeline ----
                blocks = [(sb1, hm) for sb1 in range(2) for hm in range(8)]
                sets = [_assign(nd, npool) for nd, npool
                        in zip(DVE_COUNTS, POOL_COUNTS)]
                dve_sets = [s[0] for s in sets]
                pool_sets = [s[1] for s in sets]
                state = {}

                # extra PE work interleaved into block i's q-loop:
                # extra[i][q] = list of closures (projections).
                extra = [[[] for _ in range(8)] for _ in range(16)]
                for qq in range(8):
                    extra[0][qq].append(lambda s2t=qq: emit_v_proj(s2t))
                    extra[1][qq].append(lambda s2t=8 + qq: emit_v_proj(s2t))
                for j in range(6):  # K/Q of head j+2 during block j
                    hm = j + 2
                    for g in range(4):
                        extra[j][g].append(
                            lambda mt=hm, sb=g: emit_k_group(mt, sb))
                    for g in range(2):
                        extra[j][4 + g].append(
                            lambda mt=hm, sb=g: emit_q_group(mt, sb))

                def emit_scores_half(i, s2t):
                    # fp8 DoubleRow, stride-0 slot dims: psum = 2048*q.k
                    sb1, hm = blocks[i]
                    st = ps.tile([128, 2, 512], F32, tag="st", bufs=2,
                                 name=f"st{pfx}{i}_{s2t}")
                    s1 = slice(sb1 * 512, (sb1 + 1) * 512)
                    for idx, hp in enumerate((0, 64)):
                        lh = k_t[hp:hp + 64, hm,
                                 s2t * 128:(s2t + 1) * 128].unsqueeze(
                                     1).broadcast_to((64, 2, 128))
                        rh = q_t[hp:hp + 64, hm, s1].unsqueeze(
                            1).broadcast_to((64, 2, 512))
                        nc.tensor.matmul(
                            st[:, idx, :], lh, rh,
                            start=True, stop=True, perf_mode=DR,
                            tile_position=(hp, 0))
                    exp_pair = state[i]["exp"]
                    if s2t in dve_sets[i]:
                        # fp8 bits of exp(st/16384) via one DVE op
                        nc.vector.tensor_scalar(
                            out=exp_pair[:, s2t, :, :].bitcast(U8),
                            in0=st[:], scalar1=EXP_A, scalar2=EXP_B,
                            op0=ALU.mult, op1=ALU.add)
                    elif s2t in pool_sets[i]:
                        # ACT stages PSUM->SBUF with a cheap Copy (570ns vs
                        # 1026ns for the exp), then the same bit-trick on
                        # the Pool/GPSIMD engine (which cannot read PSUM)
                        stg = sp.tile([128, 2, 512], F32, tag="stg", bufs=3,
                                      name=f"stg{pfx}{i}_{s2t}")
                        nc.scalar.activation(out=stg[:], in_=st[:],
                                             func=AF.Copy)
                        nc.gpsimd.tensor_scalar(
                            out=exp_pair[:, s2t, :, :].bitcast(U8),
                            in0=stg[:], scalar1=EXP_A, scalar2=EXP_B,
                            op0=ALU.mult, op1=ALU.add)
                    else:
                        nc.scalar.activation(
                            out=exp_pair[:, s2t, :, :], in_=st[:],
                            func=AF.Exp, scale=1.0 / 16384.0)

                def emit_pv_quad(i, q):
                    exp_pair = state[i]["exp"]
                    pvs = state[i]["pv"]
                    for idx in range(2):
                        hl = blocks[i][1] * 2 + idx
                        nc.tensor.matmul(
                            pvs[idx][:],
                            v_t[:, 2 * q:2 * q + 2, hl, :],
                            exp_pair[:, 2 * q:2 * q + 2, idx, :],
                            start=(q == 0), stop=(q == 7), perf_mode=DR)

                def emit_pv_recips(i):
                    # 1/den rows right after PV(i) stops; consumed a few
                    # iterations later so the Pool broadcast never waits.
                    for idx in range(2):
                        rec = sp.tile([1, 512], BF16, tag="rec", bufs=4,
                                      name=f"rec{pfx}{i}_{idx}")
                        with nc.allow_low_precision(
                                reason="bf16 1/den: 0.4% scale noise ok"):
                            nc.vector.reciprocal(
                                out=rec[:], in_=state[i]["pv"][idx][64:65, :])
                        state[i][f"rec{idx}"] = rec

                def emit_pv_finish(i, idx):
                    # Broadcast 1/den across partitions, then DVE
                    # multiplies: ctx = pv/den.
                    sb1, hm = blocks[i]
                    s1 = slice(sb1 * 512, (sb1 + 1) * 512)
                    hp = (0, 64)[idx]
                    pv = state[i]["pv"][idx]
                    if USE_POOL_BCAST:
                        rbc = sp.tile([64, 512], BF16, tag="rbc", bufs=2,
                                      name=f"rbc{pfx}{i}_{idx}")
                        nc.gpsimd.partition_broadcast(
                            rbc[:], state[i][f"rec{idx}"][:])
                        nc.vector.tensor_mul(
                            out=ctx_t[hp:hp + 64, hm, s1],
                            in0=pv[0:64, :], in1=rbc[:])
                    else:
                        bcp = ps.tile([64, 512], F32, tag="mm", bufs=2,
                                      name=f"bcp{pfx}{i}_{idx}")
                        nc.tensor.matmul(bcp[:], ones_row[:],
                                         state[i][f"rec{idx}"][:],
                                         start=True, stop=True)
                        nc.vector.tensor_mul(
                            out=ctx_t[hp:hp + 64, hm, s1],
                            in0=pv[0:64, :], in1=bcp[:])

                def prefetch_xrs(sb1):
                    xrs = [ep.tile([128, E], BF16, tag="xr", bufs=4,
                                   name=f"xr{pfx}{sb1 * 4 + ti}")
                           for ti in range(4)]
                    for ti in range(4):
                        rows = slice((sb1 * 4 + ti) * 128,
                                     (sb1 * 4 + ti + 1) * 128)
                        nc.sync.dma_start(out=xrs[ti][:],
                                          in_=x_res.ap()[rows, :])
                    return xrs

                g1_xrs = []

                def wo_ln_tile_closures(sb1):
                    # sb1=0 (mid-stream): batched rstd + LN apply on the
                    # idle Pool engine. sb1=1 (tail): per-tile A->rstd->B
                    # chains on DVE so output DMAs start asap (no 4-tile
                    # barrier while ACT/DVE drain).
                    gmv = ep.tile([128, 4, 2], F32, tag="gmv",
                                  name=f"gmv{pfx}{sb1}")
                    rstd_g = ep.tile([128, 4], F32, tag="rstdg",
                                     name=f"rstdg{pfx}{sb1}")
                    vs = [ep.tile([128, E], BF16, tag="v", bufs=5,
                                  name=f"v{pfx}{sb1 * 4 + ti}")
                          for ti in range(4)]
                    # residual rows prefetched (group 1: during
                    # block 15, so the tail chain never waits the DMA)
                    xrs = g1_xrs if (sb1 == 1 and g1_xrs) \
                        else prefetch_xrs(sb1)
                    stats = [ep.tile([128, 2, 6], F32, tag="stats", bufs=4,
                                     name=f"stats{pfx}{sb1 * 4 + ti}")
                             for ti in range(4)]
                    cl = []
                    for ti in range(4):
                        for eb in range(2):
                            cl.append(lambda ti=ti, eb=eb: emit_wo_half_a(
                                sb1 * 4 + ti, ti, eb, gmv, vs[ti], xrs[ti],
                                stats[ti]))
                        if sb1 == 1 and ti % 2 == 1:
                            # pair-batched rstd (each Ln<->Exp flip costs an
                            # ACT table reload) then LN-apply both tiles;
                            # first pair on Pool (idle at the tail), last
                            # pair on DVE (shortest final chain)
                            eng = nc.gpsimd if ti == 1 else nc.vector
                            cl.append(lambda t0=ti - 1: emit_rstd_dve(
                                gmv, rstd_g, t0, 2, sb1))
                            for tj in (ti - 1, ti):
                                cl.append(lambda tj=tj, eng=eng:
                                          emit_ln_tile_b(
                                              sb1 * 4 + tj, tj, gmv, rstd_g,
                                              vs[tj], eng))
                    if sb1 == 0:
                        cl.insert(8, lambda: None)  # spacing no-op
                        cl.append(lambda: emit_rstd_dve(gmv, rstd_g, 0, 4,
                                                        sb1))
                        cl.extend([lambda ti=ti: emit_ln_tile_b(
                            sb1 * 4 + ti, ti, gmv, rstd_g, vs[ti],
                            nc.vector) for ti in range(4)])
                    return cl

                def emit_wo_half_a(st_i, ti, eb, gmv, v, xr, stats):
                    p = ps.tile([128, 512], F32, tag="mm", bufs=2,
                                name=f"wop{pfx}{st_i}_{eb}")
                    for m in range(4):
                        nc.tensor.matmul(
                            p[:],
                            ctx_t[:, 2 * m:2 * m + 2,
                                  st_i * 128:(st_i + 1) * 128],
                            wo_t[:, 2 * m:2 * m + 2,
                                 eb * 512:(eb + 1) * 512],
                            start=(m == 0), stop=(m == 3), perf_mode=DR)
                    # v = psum/64 + x_res'  (ctx unscaled, WO host-scale 64)
                    nc.vector.scalar_tensor_tensor(
                        out=v[:, eb * 512:(eb + 1) * 512], in0=p[:],
                        scalar=1.0 / 64.0,
                        in1=xr[:, eb * 512:(eb + 1) * 512],
                        op0=ALU.mult, op1=ALU.add)
                    nc.vector.bn_stats(out=stats[:, eb, :],
                                       in_=v[:, eb * 512:(eb + 1) * 512])
                    if eb == 1:
                        nc.vector.bn_aggr(out=gmv[:, ti, :], in_=stats[:])

                def emit_rstd_dve(gmv, rstd_g, t0, n, sb1):
                    # rstd = rsqrt(var) via the int bit trick + 2 Newton
                    # steps, all tiny [128,n] DVE ops: keeps ACT on the
                    # softmax Exp table (each Ln/Exp flip reloads a table,
                    # 1.28us) and off the tail critical path.
                    I32 = mybir.dt.int32
                    var = gmv[:, t0:t0 + n, 1]
                    w = sp.tile([128, 4], I32, tag="rsq_i",
                                name=f"rsqi{pfx}{sb1}_{t0}")[:, 0:n]
                    nc.vector.tensor_scalar(
                        out=w, in0=var.bitcast(I32),
                        scalar1=1, scalar2=None,
                        op0=ALU.arith_shift_right)
                    nc.vector.tensor_scalar(
                        out=w, in0=w, scalar1=-1, op0=ALU.mult,
                        scalar2=0x5f3759df, op1=ALU.add)
                    y = w.bitcast(F32)
                    t = sp.tile([128, 4], F32, tag="rsq_t",
                                name=f"rsqt{pfx}{sb1}_{t0}")[:, 0:n]
                    for it in range(2):
                        dst = rstd_g[:, t0:t0 + n] if it == 1 else None
                        nc.vector.tensor_mul(out=t, in0=y, in1=y)
                        nc.vector.tensor_mul(out=t, in0=t, in1=var)
                        nc.vector.tensor_scalar(
                            out=t, in0=t, scalar1=-0.5, op0=ALU.mult,
                            scalar2=1.5, op1=ALU.add)
                        if it == 0:
                            y2 = sp.tile([128, 4], F32, tag="rsq_y",
                                         name=f"rsqy{pfx}{sb1}_{t0}")[:, 0:n]
                            nc.vector.tensor_mul(out=y2, in0=y, in1=t)
                            y = y2
                        else:
                            nc.vector.tensor_mul(out=dst, in0=y, in1=t)

                def emit_ln_tile_b(st_i, ti, gmv, rstd_g, v, eng):
                    rows = slice(st_i * 128, (st_i + 1) * 128)
                    u = v  # in-place LN apply, bf16 (4x DVE mode on vector)
                    eng.tensor_scalar(
                        out=u[:], in0=v[:],
                        scalar1=gmv[:, ti, 0:1], scalar2=rstd_g[:, ti:ti + 1],
                        op0=ALU.subtract, op1=ALU.mult)
                    eng.tensor_mul(out=u[:], in0=u[:], in1=lnw_bc[:])
                    eng.tensor_add(out=u[:], in0=u[:], in1=lnb_bc[:])
                    # issue output stores from the Pool queue: on the sync
                    # queue they head-of-line block the next rep's input
                    # DMA prefetch at the unroll boundary
                    nc.gpsimd.dma_start(out=out_half.ap()[rows, :], in_=u[:])

                wo_queue = []
                for i in range(len(blocks) + 1):
                    if i < len(blocks):
                        state[i] = {
                            "exp": ap_pool.tile([128, 16, 2, 512], FP8,
                                                tag="exp", bufs=2,
                                                name=f"exp{pfx}{i}"),
                            "pv": [ps.tile([65, 512], F32, tag="pv", bufs=2,
                                           name=f"pv{pfx}{i}_{idx}")
                                   for idx in range(2)],
                        }
                    for q in range(8):
                        if i < len(blocks):
                            emit_scores_half(i, 2 * q)
                            emit_scores_half(i, 2 * q + 1)
                            for fn in extra[i][q]:
                                fn()
                        if q == 0 and i >= 2 and i - 2 in state:
                            # both finishes before pv_quad(i-1, q0): the pv
                            # ring (2) aliases block i-2 with block i-1's
                            # accumulation running this loop
                            emit_pv_finish(i - 2, 0)
                            emit_pv_finish(i - 2, 1)
                            state.pop(i - 2)
                        if i == 15 and q == 0:
                            g1_xrs.extend(prefetch_xrs(1))
                        if i > 0:
                            emit_pv_quad(i - 1, q)
                        if wo_queue:
                            wo_queue.pop(0)()
                    if i > 0:
                        emit_pv_recips(i - 1)
                        # ctx for blocks[i-2] completed this loop; enqueue
                        # the WO/LN group once its last head's ctx is done
                        if i >= 2 and blocks[i - 2][1] == 7:
                            wo_queue.extend(
                                wo_ln_tile_closures(blocks[i - 2][0]))
                last = len(blocks) - 1
                emit_pv_finish(last, 0)
                emit_pv_finish(last, 1)
                state.pop(last)
                wo_queue.extend(wo_ln_tile_closures(1))
                for fn in wo_queue:
                    fn()

    nc.finalize()
    return nc


def _prepare_in_maps(inputs):
    f8 = ml_dtypes.float8_e4m3
    bf = ml_dtypes.bfloat16
    f32 = np.float32
    x = np.ascontiguousarray(inputs["input_tensor"], dtype=f32)
    mask = np.ascontiguousarray(inputs["mask"], dtype=f32)
    WS = 64.0    # host weight pre-scale into fp8 range (V, O)
    WSQK = 32.0  # Q/K scale: 32*q stays under fp8 e4m3 max (240)
    res_bias = (np.asarray(inputs["WO_b"], f32)
                + np.asarray(inputs["WV_b"], f32)
                @ np.asarray(inputs["WO_w"], f32).T).reshape(1, E)
    in_maps = []
    for c in range(N_CORES):
        b, hc = divmod(c, 2)
        m = {
            "xT": np.ascontiguousarray(x[b].T).astype(f8),
            "xqT": np.ascontiguousarray(
                x[b, hc * SQ:(hc + 1) * SQ].T).astype(f8),
            "x_res": (x[b, hc * SQ:(hc + 1) * SQ] + res_bias).astype(bf),
            "wqT": np.ascontiguousarray(
                inputs["WQ_w"].T * WSQK).astype(f8),
            "wkT": np.ascontiguousarray(
                inputs["WK_w"].T * WSQK).astype(f8),
            "wvT": np.ascontiguousarray(
                inputs["WV_w"].T * WS).astype(f8),
            "woT": np.ascontiguousarray(
                inputs["WO_w"].T * WS).astype(f8),
            "bq": np.ascontiguousarray(
                (np.asarray(inputs["WQ_b"], f32) * WSQK).reshape(8, 128).T),
            "bk": np.ascontiguousarray(
                (np.asarray(inputs["WK_b"], f32) * WSQK).reshape(8, 128).T),
            "em32": np.ascontiguousarray(
                32.0 * np.exp(mask[b, 0, 0]).reshape(16, 128).T.astype(f32)),
            "ln_w_row": np.asarray(
                inputs["ln_w"], f32).reshape(1, E).astype(bf),
            "ln_b_row": np.asarray(
                inputs["ln_b"], f32).reshape(1, E).astype(bf),
        }
        in_maps.append({k: np.ascontiguousarray(v) for k, v in m.items()})
    return in_maps


def _run(inputs, trace=False):
    from concourse.bass_utils import run_bass_kernel_spmd

    if "nc" not in _CACHE:
        _CACHE["nc"] = _build_nc()
    in_maps = _prepare_in_maps(inputs)
    res = run_bass_kernel_spmd(_CACHE["nc"], in_maps, list(range(N_CORES)),
                               trace=trace)
    out = np.empty((B, S, E), np.float32)
    for c in range(N_CORES):
        b, hc = divmod(c, 2)
        out[b, hc * SQ:(hc + 1) * SQ] = res.results[c]["out_half"].astype(
            np.float32)
    return out, res


def kernel(**inputs):
    out, _ = _run(inputs, trace=False)
    return out
